# revision 4
# baseline (speedup 1.0000x reference)
"""Trainium2 Bass kernel for nn_DiceLoss_11038065951148.

Reference semantics: cm[t,p] += (t==p)  -> only the diagonal accumulates, so
tp[c] = #{i : pred_i == target_i == c}; fn = fp = 0 exactly.
dice = mean_{c=1..3} 2*tp/(2*tp + 1e-6); loss = balance * (1 - dice**0.75).

Kernel strategy (memory-bound streaming, data-parallel over 8 cores):
  - shard the [1, N] int32 label arrays into 8 contiguous chunks of
    N/8 = 2,097,152 elements = [128 partitions, 16384]; pred/targ are
    interleaved per partition row on the host so each tile is ONE
    contiguous DMA (the compute ISA structs have very few sync-wait slots)
  - per tile: DVE-only compute.
      u = 4*targ + pred via one fused scalar_tensor_tensor (bf16;
      u == 5c  <=>  pred == targ == c), then three
      tensor_scalar(is_equal, accum_out) counts for u == 5, 10, 15.
    is_equal on bf16 runs in the DVE 4x perf mode (~0.3 ns/col) so the
    whole per-tile compute is ~0.66x of the tile's DMA time -- the vector
    engine never falls behind the DMA stream, and ACT stays idle (the old
    ACT sign-pass variant left a ~6 us ACT backlog after the last DMA).
  - ramped widths: small tiles at the start (compute starts early) and a
    small final tile (short tail: the tail is exactly the last tile's
    DVE time, ~1.2 us at 512 cols), 2 MB tiles in the middle for DMA
    efficiency.
  - one [128, 3, ntiles] f32 accumulator tile (row c-1 = count(u==5c) for
    tile i in column i; every slice written exactly once, no cross-tile
    hazards) is stored back in a single DMA; the host sums counts in
    float64, rounds to exact integers, applies the float32 dice formula.

Measured (serialized single-NEFF repeats, 8 cores): ~47 us per kernel
execution = ramp (~2 us DMA issue latency) + 16.78 MB/core HBM stream at
the ~370-380 GB/s per-core share of the chip's HBM bandwidth (~44.5 us,
the hard floor: 8 cores saturate all 4 HBM domains) + last-tile tail.
"""

import os
import sys

for _p in ("/opt/trn_rl_repo", "/opt/pypackages"):
    if _p not in sys.path:
        sys.path.insert(0, _p)

import numpy as np

# Set by the last kernel() call when DICE_TRACE=1: the BassKernelResults
# (exec_time_ns etc.) from run_bass_kernel_spmd. Used by test.py only.
last_results = None

N = 16_777_216
NCORES = 8
PER_CORE = N // NCORES  # 2,097,152
P = 128
TOT = PER_CORE // P  # 16384 elements per partition per tensor
NT = 4  # tiles per tensor per core (uniform default)
W = TOT // NT  # 4096
# ramped schedule: small tiles first (compute starts early), 2 MB tiles in
# the middle for bandwidth, small last tile for a short tail
WIDTHS_RAMP2 = (1024, 1024, 2048, 2048, 2048, 2048, 2048, 2048, 1024, 1024)
WIDTHS_RAMP12 = (1024, 1024, 2048, 2048, 2048, 2048, 2048, 2048, 1536, 512)
WIDTHS_RAMP13 = (512, 1024, 2048, 2048, 2048, 2048, 2048, 2048, 2048, 512)
WIDTHS_RAMP14 = (1024, 2048, 2048, 2048, 2048, 2048, 2048, 2048, 1024)
WIDTHS_RAMP15 = (512, 1536, 2048, 2048, 2048, 2048, 2048, 2048, 1536, 512)
WIDTHS_RAMP16 = (1024, 1024, 2048, 2048, 2048, 2048, 2048, 2048, 1024, 512, 512)
# tail-optimal under the cost model: compute for tile i starts only when
# tile i's DMA completes (+~0.9 us sem latency), so the post-stream tail
# obeys Lag = max_k [0.9us + sum_{i>=k} c_i - sum_{i>k} D_i].  A trailing
# run of 512-col tiles holds the tail at its ~2.2 us floor (c(512) < D(512)
# just barely), entered via a descending 1024/768 drain ramp; wide tiles go
# first where their compute backlog amortizes under the 46.6 us stream.
WIDTHS_OPT13 = (2048, 2048, 2048, 2048, 2048, 1024, 1024, 1024, 768, 768, 512, 512, 512)


def build(
    nt=NT,
    w=W,
    repeat=1,
    compute=True,
    widths=None,
    serialize=False,
    # classes whose count runs on ACT (sign-pair) for tiles at least this
    # wide; None = everything on DVE (default: DVE alone fits under the
    # DMA cadence with ~35% slack)
    act_cls3_min_w=None,
    # input prefetch depth on the mid tiles: absorbs compute jitter so the
    # DMA stream never stalls on a WAR slot
    io_bufs=5,
):
    import concourse.bacc as bacc
    import concourse.mybir as mybir
    from concourse._compat import axon_active
    from concourse.tile import TileContext, add_dep_helper

    nc = bacc.Bacc(
        "TRN2",
        target_bir_lowering=False,
        debug=not axon_active(),
        num_devices=NCORES,
        name="dice_hist",
    )
    if widths is None:
        widths = [w] * nt
    widths = list(widths)
    tot = sum(widths)
    nt = len(widths)
    offs = [sum(widths[:i]) for i in range(nt)]
    # pred and target interleaved per partition row so each tile is ONE dma
    # layout: [P, 2, tot]; tile i = columns [offs[i], offs[i]+widths[i])
    pt_d = nc.dram_tensor("pt", [P, 2, tot], mybir.dt.int32, kind="ExternalInput")
    # rows (middle axis): count(u == 5c) for c = 1..3, per tile. When
    # act_cls3_min_w is set, wide tiles instead put sum(sign(u-14.5)) in
    # row 3 (row 2 stays 0 from the memset); host formula stays universal.
    out_d = nc.dram_tensor("out", [P, 4, nt], mybir.dt.float32, kind="ExternalOutput")

    n_of_width = {wd: widths.count(wd) for wd in set(widths)}

    with TileContext(nc) as tc:
        with (
            tc.tile_pool(name="io", bufs=1) as io_pool,
            tc.tile_pool(name="wk", bufs=2) as wk_pool,
            tc.tile_pool(name="acc", bufs=1) as acc_pool,
        ):
            # one accumulator tile, every slice written by exactly one op
            acc_all = acc_pool.tile([P, 4, nt], mybir.dt.float32, tag="acc")
            nc.gpsimd.memset(acc_all[:], 0.0)
            accs = [acc_all[:, k, :] for k in range(3)]
            acc_sg = acc_all[:, 3, :]
            bias = None
            if act_cls3_min_w is not None:
                bias = acc_pool.tile([P, 1], mybir.dt.float32, tag="bias")
                nc.gpsimd.memset(bias[:], -14.5)
            prev_tail = None
            for _r in range(repeat):
                tail_inst = None
                for i in range(nt):
                    wd = widths[i]
                    tile2 = io_pool.tile(
                        [P, 2, wd],
                        mybir.dt.int32,
                        tag=f"pt{wd}",
                        bufs=min(n_of_width[wd], io_bufs),
                    )
                    d = nc.sync.dma_start(
                        tile2[:], pt_d[:, :, offs[i] : offs[i] + wd]
                    )
                    if serialize and prev_tail is not None:
                        add_dep_helper(
                            d.ins, prev_tail, sync=True, reason="serialize repeats"
                        )
                    if not compute:
                        continue
                    p_v = tile2[:, 0, :]
                    t_v = tile2[:, 1, :]
                    # u = 4*t + p in one fused op; u == 5c  <=>  p == t == c
                    u = wk_pool.tile([P, wd], mybir.dt.bfloat16, tag=f"u{wd}")
                    nc.vector.scalar_tensor_tensor(
                        out=u[:],
                        in0=t_v,
                        scalar=4.0,
                        in1=p_v,
                        op0=mybir.AluOpType.mult,
                        op1=mybir.AluOpType.add,
                    )
                    use_act = (
                        act_cls3_min_w is not None and wd >= act_cls3_min_w
                    )
                    for k in range(3):
                        if k == 2 and use_act:
                            continue
                        dm = wk_pool.tile([P, wd], mybir.dt.bfloat16, tag=f"dm{wd}")
                        v = nc.vector.tensor_scalar(
                            out=dm[:],
                            in0=u[:],
                            scalar1=float(5 * (k + 1)),
                            scalar2=None,
                            op0=mybir.AluOpType.is_equal,
                            op1=mybir.AluOpType.add,
                            accum_out=accs[k][:, i : i + 1],
                        )
                        tail_inst = v.ins
                    if use_act:
                        # count(u==15) on ACT: u <= 15 so sum(sign(u-14.5))
                        # = count(u==15) - count(u<15) = 2*n3 - wd*P
                        dm = wk_pool.tile([P, wd], mybir.dt.bfloat16, tag=f"dma{wd}")
                        a = nc.scalar.activation(
                            out=dm[:],
                            in_=u[:],
                            func=mybir.ActivationFunctionType.Sign,
                            bias=bias[:],
                            scale=1.0,
                            accum_out=acc_sg[:, i : i + 1],
                        )
                        tail_inst = a.ins
                prev_tail = tail_inst
            if compute:
                nc.sync.dma_start(out_d[:], acc_all[:])
            else:
                nc.gpsimd.dma_start(out_d[:], tile2[:, 0, : 4 * nt])
    nc.compile()
    return nc


DEFAULT_WIDTHS = WIDTHS_OPT13

_nc_cache = None


def _get_nc():
    global _nc_cache
    if _nc_cache is None:
        _nc_cache = build(widths=DEFAULT_WIDTHS)
    return _nc_cache


def unpack_counts(out_arr, widths=None, act_cols=0):
    """Per-core [P, 4, nt] device output -> (n1, n2, n3) float64 counts.

    act_cols = total columns whose class-3 count ran as an ACT sign pass
    (row 3); their contribution is (signsum + P*act_cols)/2. Slices not
    written stay 0 from the memset, so the formula is universal."""
    a = np.asarray(out_arr, dtype=np.float64).sum(axis=(0, 2))  # [4]
    n3 = a[2] + (a[3] + P * act_cols) / 2.0
    return a[0], a[1], n3


def _dice_from_counts(counts, balance, num_classes):
    # counts: float64 [4]; replicate the reference float32 arithmetic
    tp = counts.astype(np.float32)
    denom = (np.float32(2.0) * tp + np.float32(1e-6)).astype(np.float32)
    dice_per_class = (np.float32(2.0) * tp / denom).astype(np.float32)
    dice = np.float32(dice_per_class[1:].sum()) / np.float32(num_classes - 1)
    loss = np.float32(balance) * (np.float32(1.0) - dice ** np.float32(0.75))
    return np.float32(loss)


def kernel(**inputs):
    pred = np.ascontiguousarray(np.asarray(inputs["pred_labels"], dtype=np.int32))
    targ = np.ascontiguousarray(np.asarray(inputs["target_labels"], dtype=np.int32))
    balance = np.float32(np.asarray(inputs.get("balance", 1.0)))
    num_classes = int(np.asarray(inputs.get("num_classes", 4)))

    from concourse.bass_utils import run_bass_kernel_spmd

    nc = _get_nc()
    pred_sh = pred.reshape(NCORES, P, 1, TOT)
    targ_sh = targ.reshape(NCORES, P, 1, TOT)
    # interleave per partition row: [NCORES, P, 2, TOT]
    pt = np.concatenate([pred_sh, targ_sh], axis=2)
    in_maps = [{"pt": pt[i]} for i in range(NCORES)]
    trace = os.environ.get("DICE_TRACE", "") == "1"
    res = run_bass_kernel_spmd(
        nc, in_maps, core_ids=list(range(NCORES)), trace=trace
    )
    global last_results
    last_results = res

    counts = np.zeros(4, dtype=np.float64)
    for r in res.results:
        n1, n2, n3 = unpack_counts(r["out"], DEFAULT_WIDTHS)
        counts[1] += n1
        counts[2] += n2
        counts[3] += n3
    counts = np.rint(counts)
    return _dice_from_counts(counts, balance, num_classes)


# revision 7
# speedup vs baseline: 1.2653x; 1.2653x over previous
"""Trainium2 Bass kernel for nn_DiceLoss_11038065951148.

Reference semantics: cm[t,p] += (t==p)  -> only the diagonal accumulates, so
tp[c] = #{i : pred_i == target_i == c}; fn = fp = 0 exactly.
dice = mean_{c=1..3} 2*tp/(2*tp + 1e-6); loss = balance * (1 - dice**0.75).

Kernel strategy (memory-bound streaming, data-parallel over 8 cores):
  - shard the [1, N] int32 label arrays into 8 contiguous chunks of
    N/8 = 2,097,152 elements = [128 partitions, 16384]; pred/targ are
    interleaved per partition row on the host so each tile is ONE
    contiguous DMA (the compute ISA structs have very few sync-wait slots)
  - per tile: DVE-only compute.
      u = 4*targ + pred via one fused scalar_tensor_tensor (bf16;
      u == 5c  <=>  pred == targ == c), then three
      tensor_scalar(is_equal, accum_out) counts for u == 5, 10, 15.
    is_equal on bf16 runs in the DVE 4x perf mode (~0.3 ns/col) so the
    whole per-tile compute is ~0.66x of the tile's DMA time -- the vector
    engine never falls behind the DMA stream, and ACT stays idle (the old
    ACT sign-pass variant left a ~6 us ACT backlog after the last DMA).
  - ramped widths: small tiles at the start (compute starts early) and a
    small final tile (short tail: the tail is exactly the last tile's
    DVE time, ~1.2 us at 512 cols), 2 MB tiles in the middle for DMA
    efficiency.
  - one [128, 3, ntiles] f32 accumulator tile (row c-1 = count(u==5c) for
    tile i in column i; every slice written exactly once, no cross-tile
    hazards) is stored back in a single DMA; the host sums counts in
    float64, rounds to exact integers, applies the float32 dice formula.

Measured (serialized single-NEFF repeats, 8 cores): ~47 us per kernel
execution = ramp (~2 us DMA issue latency) + 16.78 MB/core HBM stream at
the ~370-380 GB/s per-core share of the chip's HBM bandwidth (~44.5 us,
the hard floor: 8 cores saturate all 4 HBM domains) + last-tile tail.
"""

import os
import sys

for _p in ("/opt/trn_rl_repo", "/opt/pypackages"):
    if _p not in sys.path:
        sys.path.insert(0, _p)

import numpy as np

# Set by the last kernel() call when DICE_TRACE=1: the BassKernelResults
# (exec_time_ns etc.) from run_bass_kernel_spmd. Used by test.py only.
last_results = None

N = 16_777_216
NCORES = 8
PER_CORE = N // NCORES  # 2,097,152
P = 128
TOT = PER_CORE // P  # 16384 elements per partition per tensor
NT = 4  # tiles per tensor per core (uniform default)
W = TOT // NT  # 4096
# ramped schedule: small tiles first (compute starts early), 2 MB tiles in
# the middle for bandwidth, small last tile for a short tail
WIDTHS_RAMP2 = (1024, 1024, 2048, 2048, 2048, 2048, 2048, 2048, 1024, 1024)
WIDTHS_RAMP12 = (1024, 1024, 2048, 2048, 2048, 2048, 2048, 2048, 1536, 512)
WIDTHS_RAMP13 = (512, 1024, 2048, 2048, 2048, 2048, 2048, 2048, 2048, 512)
WIDTHS_RAMP14 = (1024, 2048, 2048, 2048, 2048, 2048, 2048, 2048, 1024)
WIDTHS_RAMP15 = (512, 1536, 2048, 2048, 2048, 2048, 2048, 2048, 1536, 512)
WIDTHS_RAMP16 = (1024, 1024, 2048, 2048, 2048, 2048, 2048, 2048, 1024, 512, 512)
# tail-optimal under the cost model: compute for tile i starts only when
# tile i's DMA completes (+~0.9 us sem latency), so the post-stream tail
# obeys Lag = max_k [0.9us + sum_{i>=k} c_i - sum_{i>k} D_i].  A trailing
# run of 512-col tiles holds the tail at its ~2.2 us floor (c(512) < D(512)
# just barely), entered via a descending 1024/768 drain ramp; wide tiles go
# first where their compute backlog amortizes under the 46.6 us stream.
WIDTHS_OPT13 = (2048, 2048, 2048, 2048, 2048, 1024, 1024, 1024, 768, 768, 512, 512, 512)


def build(
    nt=NT,
    w=W,
    repeat=1,
    compute=True,
    widths=None,
    serialize=False,
    # repeat via a tc.For_i hardware loop instead of unrolling: tiny NEFF at
    # any repeat count, and the loop back-edge (drain + all-engine barrier,
    # ~2 us) serializes iterations like a fresh kernel launch would.
    # Used by test.py for high-signal timing; slightly conservative
    # (back-edge cost is counted into every repeat).
    hw_loop=False,
    # classes whose count runs on ACT (sign-pair) for tiles at least this
    # wide; None = everything on DVE (default: DVE alone fits under the
    # DMA cadence with ~35% slack)
    act_cls3_min_w=None,
    # input prefetch depth on the mid tiles: absorbs compute jitter so the
    # DMA stream never stalls on a WAR slot
    io_bufs=5,
):
    import concourse.bacc as bacc
    import concourse.mybir as mybir
    from concourse._compat import axon_active
    from concourse.tile import TileContext, add_dep_helper

    nc = bacc.Bacc(
        "TRN2",
        target_bir_lowering=False,
        debug=not axon_active(),
        num_devices=NCORES,
        name="dice_hist",
    )
    if widths is None:
        widths = [w] * nt
    widths = list(widths)
    tot = sum(widths)
    nt = len(widths)
    offs = [sum(widths[:i]) for i in range(nt)]
    # pred and target interleaved per partition row so each tile is ONE dma
    # layout: [P, 2, tot]; tile i = columns [offs[i], offs[i]+widths[i])
    pt_d = nc.dram_tensor("pt", [P, 2, tot], mybir.dt.int32, kind="ExternalInput")
    # rows (middle axis): count(u == 5c) for c = 1..3, per tile. When
    # act_cls3_min_w is set, wide tiles instead put sum(sign(u-14.5)) in
    # row 3 (row 2 stays 0 from the memset); host formula stays universal.
    out_d = nc.dram_tensor("out", [P, 4, nt], mybir.dt.float32, kind="ExternalOutput")

    n_of_width = {wd: widths.count(wd) for wd in set(widths)}

    with TileContext(nc) as tc:
        with (
            tc.tile_pool(name="io", bufs=1) as io_pool,
            tc.tile_pool(name="wk", bufs=2) as wk_pool,
            tc.tile_pool(name="acc", bufs=1) as acc_pool,
        ):
            # one accumulator tile, every slice written by exactly one op
            acc_all = acc_pool.tile([P, 4, nt], mybir.dt.float32, tag="acc")
            nc.gpsimd.memset(acc_all[:], 0.0)
            accs = [acc_all[:, k, :] for k in range(3)]
            acc_sg = acc_all[:, 3, :]
            bias = None
            if act_cls3_min_w is not None:
                bias = acc_pool.tile([P, 1], mybir.dt.float32, tag="bias")
                nc.gpsimd.memset(bias[:], -14.5)
            state = {"prev_tail": None, "tile2": None}

            def emit_body():
                tail_inst = None
                for i in range(nt):
                    wd = widths[i]
                    tile2 = io_pool.tile(
                        [P, 2, wd],
                        mybir.dt.int32,
                        tag=f"pt{wd}",
                        bufs=min(n_of_width[wd], io_bufs),
                    )
                    state["tile2"] = tile2
                    d = nc.sync.dma_start(
                        tile2[:], pt_d[:, :, offs[i] : offs[i] + wd]
                    )
                    if serialize and state["prev_tail"] is not None:
                        add_dep_helper(
                            d.ins,
                            state["prev_tail"],
                            sync=True,
                            reason="serialize repeats",
                        )
                    if not compute:
                        continue
                    p_v = tile2[:, 0, :]
                    t_v = tile2[:, 1, :]
                    # u = 4*t + p in one fused op; u == 5c  <=>  p == t == c
                    u = wk_pool.tile([P, wd], mybir.dt.bfloat16, tag=f"u{wd}")
                    nc.vector.scalar_tensor_tensor(
                        out=u[:],
                        in0=t_v,
                        scalar=4.0,
                        in1=p_v,
                        op0=mybir.AluOpType.mult,
                        op1=mybir.AluOpType.add,
                    )
                    use_act = (
                        act_cls3_min_w is not None and wd >= act_cls3_min_w
                    )
                    for k in range(3):
                        if k == 2 and use_act:
                            continue
                        dm = wk_pool.tile([P, wd], mybir.dt.bfloat16, tag=f"dm{wd}")
                        v = nc.vector.tensor_scalar(
                            out=dm[:],
                            in0=u[:],
                            scalar1=float(5 * (k + 1)),
                            scalar2=None,
                            op0=mybir.AluOpType.is_equal,
                            op1=mybir.AluOpType.add,
                            accum_out=accs[k][:, i : i + 1],
                        )
                        tail_inst = v.ins
                    if use_act:
                        # count(u==15) on ACT: u <= 15 so sum(sign(u-14.5))
                        # = count(u==15) - count(u<15) = 2*n3 - wd*P
                        dm = wk_pool.tile([P, wd], mybir.dt.bfloat16, tag=f"dma{wd}")
                        a = nc.scalar.activation(
                            out=dm[:],
                            in_=u[:],
                            func=mybir.ActivationFunctionType.Sign,
                            bias=bias[:],
                            scale=1.0,
                            accum_out=acc_sg[:, i : i + 1],
                        )
                        tail_inst = a.ins
                state["prev_tail"] = tail_inst

            if hw_loop and repeat > 1:
                with tc.For_i(0, repeat, 1):
                    emit_body()
            else:
                for _r in range(repeat):
                    emit_body()
            if compute:
                nc.sync.dma_start(out_d[:], acc_all[:])
            else:
                nc.gpsimd.dma_start(out_d[:], state["tile2"][:, 0, : 4 * nt])
    nc.compile()
    return nc


DEFAULT_WIDTHS = WIDTHS_OPT13

_nc_cache = None


def _get_nc():
    global _nc_cache
    if _nc_cache is None:
        _nc_cache = build(widths=DEFAULT_WIDTHS)
    return _nc_cache


def unpack_counts(out_arr, widths=None, act_cols=0):
    """Per-core [P, 4, nt] device output -> (n1, n2, n3) float64 counts.

    act_cols = total columns whose class-3 count ran as an ACT sign pass
    (row 3); their contribution is (signsum + P*act_cols)/2. Slices not
    written stay 0 from the memset, so the formula is universal."""
    a = np.asarray(out_arr, dtype=np.float64).sum(axis=(0, 2))  # [4]
    n3 = a[2] + (a[3] + P * act_cols) / 2.0
    return a[0], a[1], n3


def _dice_from_counts(counts, balance, num_classes):
    # counts: float64 [4]; replicate the reference float32 arithmetic
    tp = counts.astype(np.float32)
    denom = (np.float32(2.0) * tp + np.float32(1e-6)).astype(np.float32)
    dice_per_class = (np.float32(2.0) * tp / denom).astype(np.float32)
    dice = np.float32(dice_per_class[1:].sum()) / np.float32(num_classes - 1)
    loss = np.float32(balance) * (np.float32(1.0) - dice ** np.float32(0.75))
    return np.float32(loss)


def kernel(**inputs):
    pred = np.ascontiguousarray(np.asarray(inputs["pred_labels"], dtype=np.int32))
    targ = np.ascontiguousarray(np.asarray(inputs["target_labels"], dtype=np.int32))
    balance = np.float32(np.asarray(inputs.get("balance", 1.0)))
    num_classes = int(np.asarray(inputs.get("num_classes", 4)))

    from concourse.bass_utils import run_bass_kernel_spmd

    nc = _get_nc()
    pred_sh = pred.reshape(NCORES, P, 1, TOT)
    targ_sh = targ.reshape(NCORES, P, 1, TOT)
    # interleave per partition row: [NCORES, P, 2, TOT]
    pt = np.concatenate([pred_sh, targ_sh], axis=2)
    in_maps = [{"pt": pt[i]} for i in range(NCORES)]
    trace = os.environ.get("DICE_TRACE", "") == "1"
    res = run_bass_kernel_spmd(
        nc, in_maps, core_ids=list(range(NCORES)), trace=trace
    )
    global last_results
    last_results = res

    counts = np.zeros(4, dtype=np.float64)
    for r in res.results:
        n1, n2, n3 = unpack_counts(r["out"], DEFAULT_WIDTHS)
        counts[1] += n1
        counts[2] += n2
        counts[3] += n3
    counts = np.rint(counts)
    return _dice_from_counts(counts, balance, num_classes)


# revision 10
# speedup vs baseline: 1.9138x; 1.5125x over previous
"""Trainium2 Bass kernel for nn_DiceLoss_11038065951148.

Reference semantics: cm[t,p] += (t==p)  -> only the diagonal accumulates, so
tp[c] = #{i : pred_i == target_i == c}; fn = fp = 0 exactly.
dice = mean_{c=1..3} 2*tp/(2*tp + 1e-6); loss = balance * (1 - dice**0.75).

Kernel strategy (memory-bound streaming, data-parallel over 8 cores):
  - shard the [1, N] int32 label arrays into 8 contiguous chunks of
    N/8 = 2,097,152 elements = [128 partitions, 16384]; pred/targ are
    interleaved per partition row on the host so each tile is ONE
    contiguous DMA.
  - per tile: u = 4*targ + pred on DVE (fused scalar_tensor_tensor, bf16;
    u == 5c  <=>  pred == targ == c), then the three class counts are
    split across DVE and ACT. HW-measured: every elementwise op on this
    data runs ~1.06 ns/col on either engine (DVE fast modes do not engage
    with accum_out), so the old Sign-pair extraction of the middle class
    (2 ACT ops) made ACT the bottleneck. Key trick: ACT's Derivative_Erf
    table is a scaled Gaussian that evaluates to EXACTLY 1.1283792 (f32,
    2/sqrt(pi)) at x=0 and EXACTLY 0 for |x| >= 12 on real HW, so ONE
    activation op per class counts it:  sum(Derivative_Erf(12*(u-5c)))
    = c0 * n_c  exactly (accum_out accumulates the pre-rounding f32
    value; each per-tile-per-partition slot holds k*c0 with k <= 2048,
    recovered exactly by rint(slot/c0) on the host).
    Per-tile menus (policy-chosen): M1 = DVE{stt,eq5} + ACT{derf10,
    derf15} (balanced, ~2 units each, both engines ~86% of the DMA
    cadence); M2 = DVE{stt,eq5,eq10} + ACT{derf15}; M3 = all-DVE;
    M4 = DVE{stt} + ACT{derf5,derf10,derf15}.
  - schedule: 2 MB mid tiles for DMA efficiency, a descending tail
    (1536,1536,1024,1024,1024-M2) that minimizes the post-stream lag
    max_k [0.9us + compute-chain(k..end) - dma(k+1..end)] under the
    HW-calibrated cost model (the serial stt->ACT handoff dominates the
    last tile's chain).
  - one [128, 6, ntiles] f32 accumulator tile (rows: eq5, eq10, eq15,
    Sderf10, Sderf15, Sderf5; each slice written at most once per
    repeat) is stored back in one DMA; the host rints each derf slot to
    exact counts, sums in float64, applies the float32 dice formula.

Measured via serialized For_i hardware-loop repeats (the back-edge drain
+ all-engine barrier between iterations emulates a fresh launch and its
~2-3 us cost is charged to every repeat, so this is conservative): the
16.78 MB/core HBM stream floor alone measures ~52 us on this estimator;
the full kernel adds only the last-tile compute tail.
"""

import os
import sys

for _p in ("/opt/trn_rl_repo", "/opt/pypackages"):
    if _p not in sys.path:
        sys.path.insert(0, _p)

import numpy as np

# Set by the last kernel() call when DICE_TRACE=1: the BassKernelResults
# from run_bass_kernel_spmd. Used by test.py only.
last_results = None

N = 16_777_216
NCORES = 8
PER_CORE = N // NCORES  # 2,097,152
P = 128
TOT = PER_CORE // P  # 16384 elements per partition per tensor

# Derivative_Erf table value at x=0 (HW-verified f32: exactly this for
# every hit; exactly 0.0 for |x| >= 12, i.e. any u one integer step away
# at scale 12).
DERF_C0 = np.float32(2.0 / np.sqrt(np.pi))
DERF_SCALE = 12.0

WIDTHS_RAMP2 = (1024, 1024, 2048, 2048, 2048, 2048, 2048, 2048, 1024, 1024)
WIDTHS_OPT13 = (2048, 2048, 2048, 2048, 2048, 1024, 1024, 1024, 768, 768, 512, 512, 512)
# DP-optimal two-engine tail under the HW cost model (lag ~5.6 us vs
# ~8.2 us for uniform 2048s): descending drain, last tile DVE-heavy M2
WIDTHS_TAPER10 = (2048, 2048, 2048, 2048, 2048, 1536, 1536, 1024, 1024, 1024)


def policy_m1(wd, i, nt):
    return "M1"


def policy_taper(wd, i, nt):
    return "M2" if i == nt - 1 else "M1"


POLICIES = {
    "m1": policy_m1,
    "taper": policy_taper,
}

# menu -> (eq classes on DVE, derf classes on ACT); classes are 1,2,3
MENUS = {
    "M1": ((1,), (2, 3)),
    "M2": ((1, 2), (3,)),
    "M3": ((1, 2, 3), ()),
    "M4": ((), (1, 2, 3)),
}


def build(
    repeat=1,
    compute=True,
    widths=None,
    serialize=False,
    # repeat via a tc.For_i hardware loop: tiny NEFF at any repeat count;
    # the loop back-edge (drain + all-engine barrier) serializes
    # iterations like a fresh kernel launch. Used by test.py for
    # high-signal timing (slightly conservative).
    hw_loop=False,
    policy="taper",
    io_bufs=5,
):
    import concourse.bacc as bacc
    import concourse.mybir as mybir
    from concourse._compat import axon_active
    from concourse.tile import TileContext, add_dep_helper

    nc = bacc.Bacc(
        "TRN2",
        target_bir_lowering=False,
        debug=not axon_active(),
        num_devices=NCORES,
        name="dice_hist",
    )
    if widths is None:
        widths = WIDTHS_TAPER10
    widths = list(widths)
    tot = sum(widths)
    nt = len(widths)
    offs = [sum(widths[:i]) for i in range(nt)]
    pol = POLICIES[policy] if isinstance(policy, str) else policy
    # pred and target interleaved per partition row so each tile is ONE dma
    # layout: [P, 2, tot]; tile i = columns [offs[i], offs[i]+widths[i])
    pt_d = nc.dram_tensor("pt", [P, 2, tot], mybir.dt.int32, kind="ExternalInput")
    # acc rows: 0=eq5, 1=eq10, 2=eq15, 3=Sderf10, 4=Sderf15, 5=Sderf5
    out_d = nc.dram_tensor("out", [P, 6, nt], mybir.dt.float32, kind="ExternalOutput")

    n_of_width = {wd: widths.count(wd) for wd in set(widths)}

    with TileContext(nc) as tc:
        with (
            tc.tile_pool(name="io", bufs=1) as io_pool,
            tc.tile_pool(name="wk", bufs=2) as wk_pool,
            tc.tile_pool(name="acc", bufs=1) as acc_pool,
        ):
            acc_all = acc_pool.tile([P, 6, nt], mybir.dt.float32, tag="acc")
            nc.gpsimd.memset(acc_all[:], 0.0)
            # derf biases: -scale*5c so Derivative_Erf peaks at u == 5c
            biases = {}
            for c in (1, 2, 3):
                b = acc_pool.tile([P, 1], mybir.dt.float32, tag=f"bias{c}")
                nc.gpsimd.memset(b[:], -DERF_SCALE * 5.0 * c)
                biases[c] = b
            state = {"prev_tail": None, "tile2": None}

            def emit_body():
                tail_inst = None
                for i in range(nt):
                    wd = widths[i]
                    eq_classes, derf_classes = MENUS[pol(wd, i, nt)]
                    tile2 = io_pool.tile(
                        [P, 2, wd],
                        mybir.dt.int32,
                        tag=f"pt{wd}",
                        bufs=min(n_of_width[wd], io_bufs),
                    )
                    state["tile2"] = tile2
                    d = nc.sync.dma_start(
                        tile2[:], pt_d[:, :, offs[i] : offs[i] + wd]
                    )
                    if serialize and state["prev_tail"] is not None:
                        add_dep_helper(
                            d.ins,
                            state["prev_tail"],
                            sync=True,
                            reason="serialize repeats",
                        )
                    if not compute:
                        continue
                    p_v = tile2[:, 0, :]
                    t_v = tile2[:, 1, :]
                    # u = 4*t + p in one fused op; u == 5c  <=>  p == t == c
                    u = wk_pool.tile([P, wd], mybir.dt.bfloat16, tag=f"u{wd}")
                    nc.vector.scalar_tensor_tensor(
                        out=u[:],
                        in0=t_v,
                        scalar=4.0,
                        in1=p_v,
                        op0=mybir.AluOpType.mult,
                        op1=mybir.AluOpType.add,
                    )
                    for c in eq_classes:
                        dm = wk_pool.tile([P, wd], mybir.dt.bfloat16, tag=f"dm{wd}")
                        v = nc.vector.tensor_scalar(
                            out=dm[:],
                            in0=u[:],
                            scalar1=float(5 * c),
                            scalar2=None,
                            op0=mybir.AluOpType.is_equal,
                            op1=mybir.AluOpType.add,
                            accum_out=acc_all[:, c - 1, i : i + 1],
                        )
                        tail_inst = v.ins
                    # derf acc rows: class2->row3, class3->row4, class1->row5
                    derf_row = {2: 3, 3: 4, 1: 5}
                    for c in derf_classes:
                        dm = wk_pool.tile([P, wd], mybir.dt.bfloat16, tag=f"dma{wd}")
                        a = nc.scalar.activation(
                            out=dm[:],
                            in_=u[:],
                            func=mybir.ActivationFunctionType.Derivative_Erf,
                            bias=biases[c][:],
                            scale=DERF_SCALE,
                            accum_out=acc_all[:, derf_row[c], i : i + 1],
                        )
                        tail_inst = a.ins
                state["prev_tail"] = tail_inst

            if hw_loop and repeat > 1:
                with tc.For_i(0, repeat, 1):
                    emit_body()
            else:
                for _r in range(repeat):
                    emit_body()
            if compute:
                nc.sync.dma_start(out_d[:], acc_all[:])
            else:
                nc.gpsimd.dma_start(out_d[:], state["tile2"][:, 0, : 6 * nt])
    nc.compile()
    return nc


DEFAULT_WIDTHS = WIDTHS_TAPER10
DEFAULT_POLICY = "taper"

_nc_cache = None


def _get_nc():
    global _nc_cache
    if _nc_cache is None:
        _nc_cache = build(widths=DEFAULT_WIDTHS, policy=DEFAULT_POLICY)
    return _nc_cache


def unpack_counts(out_arr):
    """Per-core [P, 6, nt] device output -> (n1, n2, n3) float64 counts.

    eq rows (0-2) hold exact f32 integers. derf rows (3-5) hold k*c0 per
    slot (k <= tile width, c0 = 2/sqrt(pi)); rint per slot recovers k
    exactly (f32 accumulation drift per slot is << 0.5)."""
    a = np.asarray(out_arr, dtype=np.float64)  # [P, 6, nt]
    eq = a[:, 0:3, :].sum(axis=(0, 2))  # [3] classes 1,2,3
    derf = np.rint(a[:, 3:6, :] / np.float64(DERF_C0)).sum(axis=(0, 2))  # rows 3,4,5
    n1 = eq[0] + derf[2]  # row5 = class1
    n2 = eq[1] + derf[0]  # row3 = class2
    n3 = eq[2] + derf[1]  # row4 = class3
    return n1, n2, n3


def _dice_from_counts(counts, balance, num_classes):
    # counts: float64 [4]; replicate the reference float32 arithmetic
    tp = counts.astype(np.float32)
    denom = (np.float32(2.0) * tp + np.float32(1e-6)).astype(np.float32)
    dice_per_class = (np.float32(2.0) * tp / denom).astype(np.float32)
    dice = np.float32(dice_per_class[1:].sum()) / np.float32(num_classes - 1)
    loss = np.float32(balance) * (np.float32(1.0) - dice ** np.float32(0.75))
    return np.float32(loss)


def kernel(**inputs):
    pred = np.ascontiguousarray(np.asarray(inputs["pred_labels"], dtype=np.int32))
    targ = np.ascontiguousarray(np.asarray(inputs["target_labels"], dtype=np.int32))
    balance = np.float32(np.asarray(inputs.get("balance", 1.0)))
    num_classes = int(np.asarray(inputs.get("num_classes", 4)))

    from concourse.bass_utils import run_bass_kernel_spmd

    nc = _get_nc()
    pred_sh = pred.reshape(NCORES, P, 1, TOT)
    targ_sh = targ.reshape(NCORES, P, 1, TOT)
    # interleave per partition row: [NCORES, P, 2, TOT]
    pt = np.concatenate([pred_sh, targ_sh], axis=2)
    in_maps = [{"pt": pt[i]} for i in range(NCORES)]
    trace = os.environ.get("DICE_TRACE", "") == "1"
    res = run_bass_kernel_spmd(
        nc, in_maps, core_ids=list(range(NCORES)), trace=trace
    )
    global last_results
    last_results = res

    counts = np.zeros(4, dtype=np.float64)
    for r in res.results:
        n1, n2, n3 = unpack_counts(r["out"])
        counts[1] += n1
        counts[2] += n2
        counts[3] += n3
    counts = np.rint(counts)
    return _dice_from_counts(counts, balance, num_classes)


# revision 13
# speedup vs baseline: 1.9399x; 1.0137x over previous
"""Trainium2 Bass kernel for nn_DiceLoss_11038065951148.

Reference semantics: cm[t,p] += (t==p)  -> only the diagonal accumulates, so
tp[c] = #{i : pred_i == target_i == c}; fn = fp = 0 exactly.
dice = mean_{c=1..3} 2*tp/(2*tp + 1e-6); loss = balance * (1 - dice**0.75).

Kernel strategy (memory-bound streaming, data-parallel over 8 cores):
  - shard the [1, N] int32 label arrays into 8 contiguous chunks of
    N/8 = 2,097,152 elements = [128 partitions, 16384]; pred/targ are
    interleaved per partition row on the host so each tile is ONE
    contiguous DMA.
  - per tile: u = 4*targ + pred on DVE (fused scalar_tensor_tensor, bf16;
    u == 5c  <=>  pred == targ == c), then the three class counts are
    split across DVE and ACT. HW-measured: every elementwise op on this
    data runs ~1.06 ns/col on either engine (DVE fast modes do not engage
    with accum_out), so the old Sign-pair extraction of the middle class
    (2 ACT ops) made ACT the bottleneck. Key trick: ACT's Derivative_Erf
    table is a scaled Gaussian that evaluates to EXACTLY 1.1283792 (f32,
    2/sqrt(pi)) at x=0 and EXACTLY 0 for |x| >= 12 on real HW, so ONE
    activation op per class counts it:  sum(Derivative_Erf(12*(u-5c)))
    = c0 * n_c  exactly (accum_out accumulates the pre-rounding f32
    value; each per-tile-per-partition slot holds k*c0 with k <= 2048,
    recovered exactly by rint(slot/c0) on the host).
    Per-tile menus (policy-chosen): M1 = DVE{stt,eq5} + ACT{derf10,
    derf15} (balanced, ~2 units each, both engines ~86% of the DMA
    cadence); M2 = DVE{stt,eq5,eq10} + ACT{derf15}; M3 = all-DVE;
    M4 = DVE{stt} + ACT{derf5,derf10,derf15}.
  - schedule (HW A/B winner): five 2 MB mid tiles for DMA efficiency,
    then a descending 1024x3 / 512x6 drain ramp that minimizes the
    post-stream lag max_k [0.9us + compute-chain(k..end) -
    dma(k+1..end)] (the serial stt->ACT handoff dominates the last
    tile's chain; the last tile runs DVE-heavy M2).
  - one [128, 6, ntiles] f32 accumulator tile (rows: eq5, eq10, eq15,
    Sderf10, Sderf15, Sderf5; each slice written at most once per
    repeat) is stored back in one DMA; the host rints each derf slot to
    exact counts, sums in float64, applies the float32 dice formula.

Measured via serialized For_i hardware-loop repeats (the back-edge drain
+ all-engine barrier between iterations emulates a fresh launch and its
~2-3 us cost is charged to every repeat, so this is conservative): the
16.78 MB/core HBM stream floor alone measures ~52 us on this estimator;
the full kernel adds only the last-tile compute tail.
"""

import os
import sys

for _p in ("/opt/trn_rl_repo", "/opt/pypackages"):
    if _p not in sys.path:
        sys.path.insert(0, _p)

import numpy as np

# Set by the last kernel() call when DICE_TRACE=1: the BassKernelResults
# from run_bass_kernel_spmd. Used by test.py only.
last_results = None

N = 16_777_216
NCORES = 8
PER_CORE = N // NCORES  # 2,097,152
P = 128
TOT = PER_CORE // P  # 16384 elements per partition per tensor

# Derivative_Erf table value at x=0 (HW-verified f32: exactly this for
# every hit; exactly 0.0 for |x| >= 12, i.e. any u one integer step away
# at scale 12).
DERF_C0 = np.float32(2.0 / np.sqrt(np.pi))
DERF_SCALE = 12.0

WIDTHS_RAMP2 = (1024, 1024, 2048, 2048, 2048, 2048, 2048, 2048, 1024, 1024)
WIDTHS_OPT13 = (2048, 2048, 2048, 2048, 2048, 1024, 1024, 1024, 768, 768, 512, 512, 512)
# DP-optimal two-engine tail under the HW cost model (lag ~5.6 us vs
# ~8.2 us for uniform 2048s): descending drain, last tile DVE-heavy M2
WIDTHS_TAPER10 = (2048, 2048, 2048, 2048, 2048, 1536, 1536, 1024, 1024, 1024)
# HW A/B winner: descending 1024/512 drain ramp; the 512 run holds the
# post-stream tail near its floor while per-op fixed costs stay amortized
WIDTHS_W14 = (2048,) * 5 + (1024,) * 3 + (512,) * 6


def policy_m1(wd, i, nt):
    return "M1"


def policy_taper(wd, i, nt):
    return "M2" if i == nt - 1 else "M1"


POLICIES = {
    "m1": policy_m1,
    "taper": policy_taper,
}

# menu -> (eq classes on DVE, derf classes on ACT); classes are 1,2,3
MENUS = {
    "M1": ((1,), (2, 3)),
    "M2": ((1, 2), (3,)),
    "M3": ((1, 2, 3), ()),
    "M4": ((), (1, 2, 3)),
}


def build(
    repeat=1,
    compute=True,
    widths=None,
    serialize=False,
    # repeat via a tc.For_i hardware loop: tiny NEFF at any repeat count;
    # the loop back-edge (drain + all-engine barrier) serializes
    # iterations like a fresh kernel launch. Used by test.py for
    # high-signal timing (slightly conservative).
    hw_loop=False,
    policy="taper",
    io_bufs=5,
):
    import concourse.bacc as bacc
    import concourse.mybir as mybir
    from concourse._compat import axon_active
    from concourse.tile import TileContext, add_dep_helper

    nc = bacc.Bacc(
        "TRN2",
        target_bir_lowering=False,
        debug=not axon_active(),
        num_devices=NCORES,
        name="dice_hist",
    )
    if widths is None:
        widths = WIDTHS_W14
    widths = list(widths)
    tot = sum(widths)
    nt = len(widths)
    offs = [sum(widths[:i]) for i in range(nt)]
    pol = POLICIES[policy] if isinstance(policy, str) else policy
    # pred and target interleaved per partition row so each tile is ONE dma
    # layout: [P, 2, tot]; tile i = columns [offs[i], offs[i]+widths[i])
    pt_d = nc.dram_tensor("pt", [P, 2, tot], mybir.dt.int32, kind="ExternalInput")
    # acc rows: 0=eq5, 1=eq10, 2=eq15, 3=Sderf10, 4=Sderf15, 5=Sderf5
    out_d = nc.dram_tensor("out", [P, 6, nt], mybir.dt.float32, kind="ExternalOutput")

    n_of_width = {wd: widths.count(wd) for wd in set(widths)}

    with TileContext(nc) as tc:
        with (
            tc.tile_pool(name="io", bufs=1) as io_pool,
            tc.tile_pool(name="wk", bufs=2) as wk_pool,
            tc.tile_pool(name="acc", bufs=1) as acc_pool,
        ):
            acc_all = acc_pool.tile([P, 6, nt], mybir.dt.float32, tag="acc")
            nc.gpsimd.memset(acc_all[:], 0.0)
            # derf biases: -scale*5c so Derivative_Erf peaks at u == 5c
            biases = {}
            for c in (1, 2, 3):
                b = acc_pool.tile([P, 1], mybir.dt.float32, tag=f"bias{c}")
                nc.gpsimd.memset(b[:], -DERF_SCALE * 5.0 * c)
                biases[c] = b
            state = {"prev_tail": None, "tile2": None}

            def emit_body():
                tail_inst = None
                for i in range(nt):
                    wd = widths[i]
                    eq_classes, derf_classes = MENUS[pol(wd, i, nt)]
                    tile2 = io_pool.tile(
                        [P, 2, wd],
                        mybir.dt.int32,
                        tag=f"pt{wd}",
                        bufs=min(n_of_width[wd], io_bufs),
                    )
                    state["tile2"] = tile2
                    d = nc.sync.dma_start(
                        tile2[:], pt_d[:, :, offs[i] : offs[i] + wd]
                    )
                    if serialize and state["prev_tail"] is not None:
                        add_dep_helper(
                            d.ins,
                            state["prev_tail"],
                            sync=True,
                            reason="serialize repeats",
                        )
                    if not compute:
                        continue
                    p_v = tile2[:, 0, :]
                    t_v = tile2[:, 1, :]
                    # u = 4*t + p in one fused op; u == 5c  <=>  p == t == c
                    u = wk_pool.tile([P, wd], mybir.dt.bfloat16, tag=f"u{wd}")
                    nc.vector.scalar_tensor_tensor(
                        out=u[:],
                        in0=t_v,
                        scalar=4.0,
                        in1=p_v,
                        op0=mybir.AluOpType.mult,
                        op1=mybir.AluOpType.add,
                    )
                    for c in eq_classes:
                        dm = wk_pool.tile([P, wd], mybir.dt.bfloat16, tag=f"dm{wd}")
                        v = nc.vector.tensor_scalar(
                            out=dm[:],
                            in0=u[:],
                            scalar1=float(5 * c),
                            scalar2=None,
                            op0=mybir.AluOpType.is_equal,
                            op1=mybir.AluOpType.add,
                            accum_out=acc_all[:, c - 1, i : i + 1],
                        )
                        tail_inst = v.ins
                    # derf acc rows: class2->row3, class3->row4, class1->row5
                    derf_row = {2: 3, 3: 4, 1: 5}
                    for c in derf_classes:
                        dm = wk_pool.tile([P, wd], mybir.dt.bfloat16, tag=f"dma{wd}")
                        a = nc.scalar.activation(
                            out=dm[:],
                            in_=u[:],
                            func=mybir.ActivationFunctionType.Derivative_Erf,
                            bias=biases[c][:],
                            scale=DERF_SCALE,
                            accum_out=acc_all[:, derf_row[c], i : i + 1],
                        )
                        tail_inst = a.ins
                state["prev_tail"] = tail_inst

            if hw_loop and repeat > 1:
                with tc.For_i(0, repeat, 1):
                    emit_body()
            else:
                for _r in range(repeat):
                    emit_body()
            if compute:
                nc.sync.dma_start(out_d[:], acc_all[:])
            else:
                nc.gpsimd.dma_start(out_d[:], state["tile2"][:, 0, : 6 * nt])
    nc.compile()
    return nc


DEFAULT_WIDTHS = WIDTHS_W14
DEFAULT_POLICY = "taper"

_nc_cache = None


def _get_nc():
    global _nc_cache
    if _nc_cache is None:
        _nc_cache = build(widths=DEFAULT_WIDTHS, policy=DEFAULT_POLICY)
    return _nc_cache


def unpack_counts(out_arr):
    """Per-core [P, 6, nt] device output -> (n1, n2, n3) float64 counts.

    eq rows (0-2) hold exact f32 integers. derf rows (3-5) hold k*c0 per
    slot (k <= tile width, c0 = 2/sqrt(pi)); rint per slot recovers k
    exactly (f32 accumulation drift per slot is << 0.5)."""
    a = np.asarray(out_arr, dtype=np.float64)  # [P, 6, nt]
    eq = a[:, 0:3, :].sum(axis=(0, 2))  # [3] classes 1,2,3
    derf = np.rint(a[:, 3:6, :] / np.float64(DERF_C0)).sum(axis=(0, 2))  # rows 3,4,5
    n1 = eq[0] + derf[2]  # row5 = class1
    n2 = eq[1] + derf[0]  # row3 = class2
    n3 = eq[2] + derf[1]  # row4 = class3
    return n1, n2, n3


def _dice_from_counts(counts, balance, num_classes):
    # counts: float64 [4]; replicate the reference float32 arithmetic
    tp = counts.astype(np.float32)
    denom = (np.float32(2.0) * tp + np.float32(1e-6)).astype(np.float32)
    dice_per_class = (np.float32(2.0) * tp / denom).astype(np.float32)
    dice = np.float32(dice_per_class[1:].sum()) / np.float32(num_classes - 1)
    loss = np.float32(balance) * (np.float32(1.0) - dice ** np.float32(0.75))
    return np.float32(loss)


def kernel(**inputs):
    pred = np.ascontiguousarray(np.asarray(inputs["pred_labels"], dtype=np.int32))
    targ = np.ascontiguousarray(np.asarray(inputs["target_labels"], dtype=np.int32))
    balance = np.float32(np.asarray(inputs.get("balance", 1.0)))
    num_classes = int(np.asarray(inputs.get("num_classes", 4)))

    from concourse.bass_utils import run_bass_kernel_spmd

    nc = _get_nc()
    pred_sh = pred.reshape(NCORES, P, 1, TOT)
    targ_sh = targ.reshape(NCORES, P, 1, TOT)
    # interleave per partition row: [NCORES, P, 2, TOT]
    pt = np.concatenate([pred_sh, targ_sh], axis=2)
    in_maps = [{"pt": pt[i]} for i in range(NCORES)]
    trace = os.environ.get("DICE_TRACE", "") == "1"
    res = run_bass_kernel_spmd(
        nc, in_maps, core_ids=list(range(NCORES)), trace=trace
    )
    global last_results
    last_results = res

    counts = np.zeros(4, dtype=np.float64)
    for r in res.results:
        n1, n2, n3 = unpack_counts(r["out"])
        counts[1] += n1
        counts[2] += n2
        counts[3] += n3
    counts = np.rint(counts)
    return _dice_from_counts(counts, balance, num_classes)


# revision 19
# speedup vs baseline: 1.9510x; 1.0057x over previous
"""Trainium2 Bass kernel for nn_DiceLoss_11038065951148.

Reference semantics: cm[t,p] += (t==p)  -> only the diagonal accumulates, so
tp[c] = #{i : pred_i == target_i == c}; fn = fp = 0 exactly.
dice = mean_{c=1..3} 2*tp/(2*tp + 1e-6); loss = balance * (1 - dice**0.75).

Kernel strategy (memory-bound streaming, data-parallel over 8 cores):
  - shard the [1, N] int32 label arrays into 8 contiguous chunks of
    N/8 = 2,097,152 elements = [128 partitions, 16384]; pred/targ are
    interleaved per partition row on the host so each tile is ONE
    contiguous DMA.
  - per tile: u = 4*targ + pred on DVE (fused scalar_tensor_tensor, bf16;
    u == 5c  <=>  pred == targ == c), then the three class counts are
    split across DVE and ACT. HW-measured: every elementwise op on this
    data runs ~1.06 ns/col on either engine (DVE fast modes do not engage
    with accum_out), so the old Sign-pair extraction of the middle class
    (2 ACT ops) made ACT the bottleneck. Key trick: ACT's Derivative_Erf
    table is a scaled Gaussian that evaluates to EXACTLY 1.1283792 (f32,
    2/sqrt(pi)) at x=0 and EXACTLY 0 for |x| >= 12 on real HW, so ONE
    activation op per class counts it:  sum(Derivative_Erf(12*(u-5c)))
    = c0 * n_c  exactly (accum_out accumulates the pre-rounding f32
    value; each per-tile-per-partition slot holds k*c0 with k <= 2048,
    recovered exactly by rint(slot/c0) on the host).
    Per-tile menus (policy-chosen): M1 = DVE{stt,eq5} + ACT{derf10,
    derf15} (balanced, ~2 units each, both engines ~86% of the DMA
    cadence); M2 = DVE{stt,eq5,eq10} + ACT{derf15}; M3 = all-DVE;
    M4 = DVE{stt} + ACT{derf5,derf10,derf15}.
  - schedule (HW A/B winner): five 2 MB mid tiles for DMA efficiency,
    then a descending 1024x3 / 512x6 drain ramp that minimizes the
    post-stream lag max_k [0.9us + compute-chain(k..end) -
    dma(k+1..end)] (the serial stt->ACT handoff dominates the last
    tile's chain; the last tile runs DVE-heavy M2).
  - one [128, 6, ntiles] f32 accumulator tile (rows: eq5, eq10, eq15,
    Sderf10, Sderf15, Sderf5; each slice written at most once per
    repeat) is stored back in one DMA; the host rints each derf slot to
    exact counts, sums in float64, applies the float32 dice formula.

Measured via serialized For_i hardware-loop repeats (the back-edge drain
+ all-engine barrier between iterations emulates a fresh launch and its
~2-3 us cost is charged to every repeat, so this is conservative):
~54-55 us per execution. Same-estimator, same-session comparison: the
previous Sign-pair baseline measures ~55.7 us, and the pure DMA stream
floor (no compute at all) measures ~49 us -- the kernel runs ~5 us
(the last tiles' serial compute chains) above the HBM-bandwidth floor,
with both compute engines at ~86% of the DMA cadence mid-stream.
"""

import os
import sys

for _p in ("/opt/trn_rl_repo", "/opt/pypackages"):
    if _p not in sys.path:
        sys.path.insert(0, _p)

import numpy as np

# Set by the last kernel() call when DICE_TRACE=1: the BassKernelResults
# from run_bass_kernel_spmd. Used by test.py only.
last_results = None

N = 16_777_216
NCORES = 8
PER_CORE = N // NCORES  # 2,097,152
P = 128
TOT = PER_CORE // P  # 16384 elements per partition per tensor

# Derivative_Erf table value at x=0 (HW-verified f32: exactly this for
# every hit; exactly 0.0 for |x| >= 12, i.e. any u one integer step away
# at scale 12).
DERF_C0 = np.float32(2.0 / np.sqrt(np.pi))
DERF_SCALE = 12.0

WIDTHS_RAMP2 = (1024, 1024, 2048, 2048, 2048, 2048, 2048, 2048, 1024, 1024)
WIDTHS_OPT13 = (2048, 2048, 2048, 2048, 2048, 1024, 1024, 1024, 768, 768, 512, 512, 512)
# DP-optimal two-engine tail under the HW cost model (lag ~5.6 us vs
# ~8.2 us for uniform 2048s): descending drain, last tile DVE-heavy M2
WIDTHS_TAPER10 = (2048, 2048, 2048, 2048, 2048, 1536, 1536, 1024, 1024, 1024)
# HW A/B winner: descending 1024/512 drain ramp; the 512 run holds the
# post-stream tail near its floor while per-op fixed costs stay amortized
WIDTHS_W14 = (2048,) * 5 + (1024,) * 3 + (512,) * 6


def policy_m1(wd, i, nt):
    return "M1"


def policy_taper(wd, i, nt):
    return "M2" if i == nt - 1 else "M1"


def policy_taper2(wd, i, nt):
    return "M2" if i >= nt - 2 else "M1"


POLICIES = {
    "m1": policy_m1,
    "taper": policy_taper,
    "taper2": policy_taper2,
}

# menu -> (eq classes on DVE, derf classes on ACT); classes are 1,2,3
MENUS = {
    "M1": ((1,), (2, 3)),
    "M2": ((1, 2), (3,)),
    "M3": ((1, 2, 3), ()),
    "M4": ((), (1, 2, 3)),
}


def build(
    repeat=1,
    compute=True,
    widths=None,
    serialize=False,
    # repeat via a tc.For_i hardware loop: tiny NEFF at any repeat count;
    # the loop back-edge (drain + all-engine barrier) serializes
    # iterations like a fresh kernel launch. Used by test.py for
    # high-signal timing (slightly conservative).
    hw_loop=False,
    policy="taper",
    io_bufs=5,
    # cap on the column width of each compute op: DMA tiles wider than
    # this are processed as several compute sub-slices (own acc column
    # each), so wide DMA tiles (better stream rate) don't force wide
    # SBUF work tiles
    comp_w=None,
):
    import concourse.bacc as bacc
    import concourse.mybir as mybir
    from concourse._compat import axon_active
    from concourse.tile import TileContext, add_dep_helper

    nc = bacc.Bacc(
        "TRN2",
        target_bir_lowering=False,
        debug=not axon_active(),
        num_devices=NCORES,
        name="dice_hist",
    )
    if widths is None:
        widths = WIDTHS_W14
    widths = list(widths)
    tot = sum(widths)
    nt = len(widths)
    offs = [sum(widths[:i]) for i in range(nt)]
    pol = POLICIES[policy] if isinstance(policy, str) else policy
    # compute sub-slices per DMA tile (own acc column each)
    subs = []  # per tile: list of (col_offset_in_tile, sub_width, acc_col)
    j = 0
    for wd in widths:
        lst = []
        off = 0
        while off < wd:
            w_s = wd - off if comp_w is None else min(comp_w, wd - off)
            lst.append((off, w_s, j))
            off += w_s
            j += 1
        subs.append(lst)
    nt_sub = j
    # pred and target interleaved per partition row so each tile is ONE dma
    # layout: [P, 2, tot]; tile i = columns [offs[i], offs[i]+widths[i])
    pt_d = nc.dram_tensor("pt", [P, 2, tot], mybir.dt.int32, kind="ExternalInput")
    # acc rows: 0=eq5, 1=eq10, 2=eq15, 3=Sderf10, 4=Sderf15, 5=Sderf5
    out_d = nc.dram_tensor("out", [P, 6, nt_sub], mybir.dt.float32, kind="ExternalOutput")

    n_of_width = {wd: widths.count(wd) for wd in set(widths)}

    with TileContext(nc) as tc:
        with (
            tc.tile_pool(name="io", bufs=1) as io_pool,
            tc.tile_pool(name="wk", bufs=2) as wk_pool,
            tc.tile_pool(name="acc", bufs=1) as acc_pool,
        ):
            acc_all = acc_pool.tile([P, 6, nt_sub], mybir.dt.float32, tag="acc")
            nc.gpsimd.memset(acc_all[:], 0.0)
            # derf biases: -scale*5c so Derivative_Erf peaks at u == 5c
            biases = {}
            for c in (1, 2, 3):
                b = acc_pool.tile([P, 1], mybir.dt.float32, tag=f"bias{c}")
                nc.gpsimd.memset(b[:], -DERF_SCALE * 5.0 * c)
                biases[c] = b
            state = {"prev_tail": None, "tile2": None}

            def emit_body():
                tail_inst = None
                for i in range(nt):
                    wd = widths[i]
                    eq_classes, derf_classes = MENUS[pol(wd, i, nt)]
                    tile2 = io_pool.tile(
                        [P, 2, wd],
                        mybir.dt.int32,
                        tag=f"pt{wd}",
                        bufs=min(n_of_width[wd], io_bufs),
                    )
                    state["tile2"] = tile2
                    d = nc.sync.dma_start(
                        tile2[:], pt_d[:, :, offs[i] : offs[i] + wd]
                    )
                    if serialize and state["prev_tail"] is not None:
                        add_dep_helper(
                            d.ins,
                            state["prev_tail"],
                            sync=True,
                            reason="serialize repeats",
                        )
                    if not compute:
                        continue
                    # derf acc rows: class2->row3, class3->row4, class1->row5
                    derf_row = {2: 3, 3: 4, 1: 5}
                    for s_off, w_s, jcol in subs[i]:
                        p_v = tile2[:, 0, s_off : s_off + w_s]
                        t_v = tile2[:, 1, s_off : s_off + w_s]
                        # u = 4*t + p in one fused op; u == 5c <=> p == t == c
                        u = wk_pool.tile([P, w_s], mybir.dt.bfloat16, tag=f"u{w_s}")
                        nc.vector.scalar_tensor_tensor(
                            out=u[:],
                            in0=t_v,
                            scalar=4.0,
                            in1=p_v,
                            op0=mybir.AluOpType.mult,
                            op1=mybir.AluOpType.add,
                        )
                        for c in eq_classes:
                            dm = wk_pool.tile(
                                [P, w_s], mybir.dt.bfloat16, tag=f"dm{w_s}"
                            )
                            v = nc.vector.tensor_scalar(
                                out=dm[:],
                                in0=u[:],
                                scalar1=float(5 * c),
                                scalar2=None,
                                op0=mybir.AluOpType.is_equal,
                                op1=mybir.AluOpType.add,
                                accum_out=acc_all[:, c - 1, jcol : jcol + 1],
                            )
                            tail_inst = v.ins
                        for c in derf_classes:
                            dm = wk_pool.tile(
                                [P, w_s], mybir.dt.bfloat16, tag=f"dma{w_s}"
                            )
                            a = nc.scalar.activation(
                                out=dm[:],
                                in_=u[:],
                                func=mybir.ActivationFunctionType.Derivative_Erf,
                                bias=biases[c][:],
                                scale=DERF_SCALE,
                                accum_out=acc_all[:, derf_row[c], jcol : jcol + 1],
                            )
                            tail_inst = a.ins
                state["prev_tail"] = tail_inst

            if hw_loop and repeat > 1:
                with tc.For_i(0, repeat, 1):
                    emit_body()
            else:
                for _r in range(repeat):
                    emit_body()
            if compute:
                nc.sync.dma_start(out_d[:], acc_all[:])
            else:
                nc.gpsimd.dma_start(out_d[:], state["tile2"][:, 0, : 6 * nt])
    nc.compile()
    return nc


DEFAULT_WIDTHS = WIDTHS_W14
DEFAULT_POLICY = "taper"

_nc_cache = None


def _get_nc():
    global _nc_cache
    if _nc_cache is None:
        _nc_cache = build(widths=DEFAULT_WIDTHS, policy=DEFAULT_POLICY)
    return _nc_cache


def unpack_counts(out_arr):
    """Per-core [P, 6, nt] device output -> (n1, n2, n3) float64 counts.

    eq rows (0-2) hold exact f32 integers. derf rows (3-5) hold k*c0 per
    slot (k <= tile width, c0 = 2/sqrt(pi)); rint per slot recovers k
    exactly (f32 accumulation drift per slot is << 0.5)."""
    a = np.asarray(out_arr, dtype=np.float64)  # [P, 6, nt]
    eq = a[:, 0:3, :].sum(axis=(0, 2))  # [3] classes 1,2,3
    derf = np.rint(a[:, 3:6, :] / np.float64(DERF_C0)).sum(axis=(0, 2))  # rows 3,4,5
    n1 = eq[0] + derf[2]  # row5 = class1
    n2 = eq[1] + derf[0]  # row3 = class2
    n3 = eq[2] + derf[1]  # row4 = class3
    return n1, n2, n3


def _dice_from_counts(counts, balance, num_classes):
    # counts: float64 [4]; replicate the reference float32 arithmetic
    tp = counts.astype(np.float32)
    denom = (np.float32(2.0) * tp + np.float32(1e-6)).astype(np.float32)
    dice_per_class = (np.float32(2.0) * tp / denom).astype(np.float32)
    dice = np.float32(dice_per_class[1:].sum()) / np.float32(num_classes - 1)
    loss = np.float32(balance) * (np.float32(1.0) - dice ** np.float32(0.75))
    return np.float32(loss)


def kernel(**inputs):
    pred = np.ascontiguousarray(np.asarray(inputs["pred_labels"], dtype=np.int32))
    targ = np.ascontiguousarray(np.asarray(inputs["target_labels"], dtype=np.int32))
    balance = np.float32(np.asarray(inputs.get("balance", 1.0)))
    num_classes = int(np.asarray(inputs.get("num_classes", 4)))

    from concourse.bass_utils import run_bass_kernel_spmd

    nc = _get_nc()
    pred_sh = pred.reshape(NCORES, P, 1, TOT)
    targ_sh = targ.reshape(NCORES, P, 1, TOT)
    # interleave per partition row: [NCORES, P, 2, TOT]
    pt = np.concatenate([pred_sh, targ_sh], axis=2)
    in_maps = [{"pt": pt[i]} for i in range(NCORES)]
    trace = os.environ.get("DICE_TRACE", "") == "1"
    res = run_bass_kernel_spmd(
        nc, in_maps, core_ids=list(range(NCORES)), trace=trace
    )
    global last_results
    last_results = res

    counts = np.zeros(4, dtype=np.float64)
    for r in res.results:
        n1, n2, n3 = unpack_counts(r["out"])
        counts[1] += n1
        counts[2] += n2
        counts[3] += n3
    counts = np.rint(counts)
    return _dice_from_counts(counts, balance, num_classes)


# revision 22
# speedup vs baseline: 1.9538x; 1.0014x over previous
"""Trainium2 Bass kernel for nn_DiceLoss_11038065951148.

Reference semantics: cm[t,p] += (t==p)  -> only the diagonal accumulates, so
tp[c] = #{i : pred_i == target_i == c}; fn = fp = 0 exactly.
dice = mean_{c=1..3} 2*tp/(2*tp + 1e-6); loss = balance * (1 - dice**0.75).

Kernel strategy (memory-bound streaming, data-parallel over 8 cores):
  - shard the [1, N] int32 label arrays into 8 contiguous chunks of
    N/8 = 2,097,152 elements = [128 partitions, 16384]; pred/targ are
    interleaved per partition row on the host so each tile is ONE
    contiguous DMA.
  - per tile: u = 4*targ + pred on DVE (fused scalar_tensor_tensor, bf16;
    u == 5c  <=>  pred == targ == c), then the three class counts are
    split across DVE and ACT. HW-measured: every elementwise op on this
    data runs ~1.06 ns/col on either engine (DVE fast modes do not engage
    with accum_out), so the old Sign-pair extraction of the middle class
    (2 ACT ops) made ACT the bottleneck. Key trick: ACT's Derivative_Erf
    table is a scaled Gaussian that evaluates to EXACTLY 1.1283792 (f32,
    2/sqrt(pi)) at x=0 and EXACTLY 0 for |x| >= 12 on real HW, so ONE
    activation op per class counts it:  sum(Derivative_Erf(12*(u-5c)))
    = c0 * n_c  exactly (accum_out accumulates the pre-rounding f32
    value; each per-tile-per-partition slot holds k*c0 with k <= 2048,
    recovered exactly by rint(slot/c0) on the host).
    Per-tile menus (policy-chosen): M1 = DVE{stt,eq5} + ACT{derf10,
    derf15} (balanced, ~2 units each, both engines ~86% of the DMA
    cadence); M2 = DVE{stt,eq5,eq10} + ACT{derf15}; M3 = all-DVE;
    M4 = DVE{stt} + ACT{derf5,derf10,derf15}.
  - schedule (HW A/B winner): five 2 MB mid tiles for DMA efficiency,
    then a descending 1024x3 / 512x6 drain ramp that minimizes the
    post-stream lag max_k [0.9us + compute-chain(k..end) -
    dma(k+1..end)] (the serial stt->ACT handoff dominates the last
    tile's chain; the last tile runs DVE-heavy M2).
  - one [128, 6, ntiles] f32 accumulator tile (rows: eq5, eq10, eq15,
    Sderf10, Sderf15, Sderf5; each slice written at most once per
    repeat) is stored back in one DMA; the host rints each derf slot to
    exact counts, sums in float64, applies the float32 dice formula.

Measured via serialized For_i hardware-loop repeats (the back-edge drain
+ all-engine barrier between iterations emulates a fresh launch and its
~2-3 us cost is charged to every repeat, so this is conservative):
~53-55 us per execution (io prefetch depth 7: the 6th tail-tile DMA
no longer stalls on a WAR release, measured -1.4 us vs depth 5).
Same-estimator, same-session comparison: the
previous Sign-pair baseline measures ~55.7 us, and the pure DMA stream
floor (no compute at all) measures ~49 us -- the kernel runs ~5 us
(the last tiles' serial compute chains) above the HBM-bandwidth floor,
with both compute engines at ~86% of the DMA cadence mid-stream.
"""

import os
import sys

for _p in ("/opt/trn_rl_repo", "/opt/pypackages"):
    if _p not in sys.path:
        sys.path.insert(0, _p)

import numpy as np

# Set by the last kernel() call when DICE_TRACE=1: the BassKernelResults
# from run_bass_kernel_spmd. Used by test.py only.
last_results = None

N = 16_777_216
NCORES = 8
PER_CORE = N // NCORES  # 2,097,152
P = 128
TOT = PER_CORE // P  # 16384 elements per partition per tensor

# Derivative_Erf table value at x=0 (HW-verified f32: exactly this for
# every hit; exactly 0.0 for |x| >= 12, i.e. any u one integer step away
# at scale 12).
DERF_C0 = np.float32(2.0 / np.sqrt(np.pi))
DERF_SCALE = 12.0

WIDTHS_RAMP2 = (1024, 1024, 2048, 2048, 2048, 2048, 2048, 2048, 1024, 1024)
WIDTHS_OPT13 = (2048, 2048, 2048, 2048, 2048, 1024, 1024, 1024, 768, 768, 512, 512, 512)
# DP-optimal two-engine tail under the HW cost model (lag ~5.6 us vs
# ~8.2 us for uniform 2048s): descending drain, last tile DVE-heavy M2
WIDTHS_TAPER10 = (2048, 2048, 2048, 2048, 2048, 1536, 1536, 1024, 1024, 1024)
# HW A/B winner: descending 1024/512 drain ramp; the 512 run holds the
# post-stream tail near its floor while per-op fixed costs stay amortized
WIDTHS_W14 = (2048,) * 5 + (1024,) * 3 + (512,) * 6


def policy_m1(wd, i, nt):
    return "M1"


def policy_taper(wd, i, nt):
    return "M2" if i == nt - 1 else "M1"


def policy_taper2(wd, i, nt):
    return "M2" if i >= nt - 2 else "M1"


POLICIES = {
    "m1": policy_m1,
    "taper": policy_taper,
    "taper2": policy_taper2,
}

# menu -> (eq classes on DVE, derf classes on ACT); classes are 1,2,3
MENUS = {
    "M1": ((1,), (2, 3)),
    "M2": ((1, 2), (3,)),
    "M3": ((1, 2, 3), ()),
    "M4": ((), (1, 2, 3)),
}


def build(
    repeat=1,
    compute=True,
    widths=None,
    serialize=False,
    # repeat via a tc.For_i hardware loop: tiny NEFF at any repeat count;
    # the loop back-edge (drain + all-engine barrier) serializes
    # iterations like a fresh kernel launch. Used by test.py for
    # high-signal timing (slightly conservative).
    hw_loop=False,
    policy="taper",
    io_bufs=7,
    # work-pool depth per tag (u and mask tiles): 3 lets DVE run a tile
    # ahead of ACT's trailing derf reads at the tail without a WAR stall
    wk_bufs=2,
    # cap on the column width of each compute op: DMA tiles wider than
    # this are processed as several compute sub-slices (own acc column
    # each), so wide DMA tiles (better stream rate) don't force wide
    # SBUF work tiles
    comp_w=None,
):
    import concourse.bacc as bacc
    import concourse.mybir as mybir
    from concourse._compat import axon_active
    from concourse.tile import TileContext, add_dep_helper

    nc = bacc.Bacc(
        "TRN2",
        target_bir_lowering=False,
        debug=not axon_active(),
        num_devices=NCORES,
        name="dice_hist",
    )
    if widths is None:
        widths = WIDTHS_W14
    widths = list(widths)
    tot = sum(widths)
    nt = len(widths)
    offs = [sum(widths[:i]) for i in range(nt)]
    pol = POLICIES[policy] if isinstance(policy, str) else policy
    # compute sub-slices per DMA tile (own acc column each)
    subs = []  # per tile: list of (col_offset_in_tile, sub_width, acc_col)
    j = 0
    for wd in widths:
        lst = []
        off = 0
        while off < wd:
            w_s = wd - off if comp_w is None else min(comp_w, wd - off)
            lst.append((off, w_s, j))
            off += w_s
            j += 1
        subs.append(lst)
    nt_sub = j
    # pred and target interleaved per partition row so each tile is ONE dma
    # layout: [P, 2, tot]; tile i = columns [offs[i], offs[i]+widths[i])
    pt_d = nc.dram_tensor("pt", [P, 2, tot], mybir.dt.int32, kind="ExternalInput")
    # acc rows: 0=eq5, 1=eq10, 2=eq15, 3=Sderf10, 4=Sderf15, 5=Sderf5
    out_d = nc.dram_tensor("out", [P, 6, nt_sub], mybir.dt.float32, kind="ExternalOutput")

    n_of_width = {wd: widths.count(wd) for wd in set(widths)}

    with TileContext(nc) as tc:
        with (
            tc.tile_pool(name="io", bufs=1) as io_pool,
            tc.tile_pool(name="wk", bufs=wk_bufs) as wk_pool,
            tc.tile_pool(name="acc", bufs=1) as acc_pool,
        ):
            acc_all = acc_pool.tile([P, 6, nt_sub], mybir.dt.float32, tag="acc")
            nc.gpsimd.memset(acc_all[:], 0.0)
            # derf biases: -scale*5c so Derivative_Erf peaks at u == 5c
            biases = {}
            for c in (1, 2, 3):
                b = acc_pool.tile([P, 1], mybir.dt.float32, tag=f"bias{c}")
                nc.gpsimd.memset(b[:], -DERF_SCALE * 5.0 * c)
                biases[c] = b
            state = {"prev_tail": None, "tile2": None}

            def emit_body():
                tail_inst = None
                for i in range(nt):
                    wd = widths[i]
                    eq_classes, derf_classes = MENUS[pol(wd, i, nt)]
                    tile2 = io_pool.tile(
                        [P, 2, wd],
                        mybir.dt.int32,
                        tag=f"pt{wd}",
                        bufs=min(n_of_width[wd], io_bufs),
                    )
                    state["tile2"] = tile2
                    d = nc.sync.dma_start(
                        tile2[:], pt_d[:, :, offs[i] : offs[i] + wd]
                    )
                    if serialize and state["prev_tail"] is not None:
                        add_dep_helper(
                            d.ins,
                            state["prev_tail"],
                            sync=True,
                            reason="serialize repeats",
                        )
                    if not compute:
                        continue
                    # derf acc rows: class2->row3, class3->row4, class1->row5
                    derf_row = {2: 3, 3: 4, 1: 5}
                    for s_off, w_s, jcol in subs[i]:
                        p_v = tile2[:, 0, s_off : s_off + w_s]
                        t_v = tile2[:, 1, s_off : s_off + w_s]
                        # u = 4*t + p in one fused op; u == 5c <=> p == t == c
                        u = wk_pool.tile([P, w_s], mybir.dt.bfloat16, tag=f"u{w_s}")
                        nc.vector.scalar_tensor_tensor(
                            out=u[:],
                            in0=t_v,
                            scalar=4.0,
                            in1=p_v,
                            op0=mybir.AluOpType.mult,
                            op1=mybir.AluOpType.add,
                        )
                        for c in eq_classes:
                            dm = wk_pool.tile(
                                [P, w_s], mybir.dt.bfloat16, tag=f"dm{w_s}"
                            )
                            v = nc.vector.tensor_scalar(
                                out=dm[:],
                                in0=u[:],
                                scalar1=float(5 * c),
                                scalar2=None,
                                op0=mybir.AluOpType.is_equal,
                                op1=mybir.AluOpType.add,
                                accum_out=acc_all[:, c - 1, jcol : jcol + 1],
                            )
                            tail_inst = v.ins
                        for c in derf_classes:
                            dm = wk_pool.tile(
                                [P, w_s], mybir.dt.bfloat16, tag=f"dma{w_s}"
                            )
                            a = nc.scalar.activation(
                                out=dm[:],
                                in_=u[:],
                                func=mybir.ActivationFunctionType.Derivative_Erf,
                                bias=biases[c][:],
                                scale=DERF_SCALE,
                                accum_out=acc_all[:, derf_row[c], jcol : jcol + 1],
                            )
                            tail_inst = a.ins
                state["prev_tail"] = tail_inst

            if hw_loop and repeat > 1:
                with tc.For_i(0, repeat, 1):
                    emit_body()
            else:
                for _r in range(repeat):
                    emit_body()
            if compute:
                nc.sync.dma_start(out_d[:], acc_all[:])
            else:
                nc.gpsimd.dma_start(out_d[:], state["tile2"][:, 0, : 6 * nt])
    nc.compile()
    return nc


DEFAULT_WIDTHS = WIDTHS_W14
DEFAULT_POLICY = "taper"

_nc_cache = None


def _get_nc():
    global _nc_cache
    if _nc_cache is None:
        _nc_cache = build(widths=DEFAULT_WIDTHS, policy=DEFAULT_POLICY)
    return _nc_cache


def unpack_counts(out_arr):
    """Per-core [P, 6, nt] device output -> (n1, n2, n3) float64 counts.

    eq rows (0-2) hold exact f32 integers. derf rows (3-5) hold k*c0 per
    slot (k <= tile width, c0 = 2/sqrt(pi)); rint per slot recovers k
    exactly (f32 accumulation drift per slot is << 0.5)."""
    a = np.asarray(out_arr, dtype=np.float64)  # [P, 6, nt]
    eq = a[:, 0:3, :].sum(axis=(0, 2))  # [3] classes 1,2,3
    derf = np.rint(a[:, 3:6, :] / np.float64(DERF_C0)).sum(axis=(0, 2))  # rows 3,4,5
    n1 = eq[0] + derf[2]  # row5 = class1
    n2 = eq[1] + derf[0]  # row3 = class2
    n3 = eq[2] + derf[1]  # row4 = class3
    return n1, n2, n3


def _dice_from_counts(counts, balance, num_classes):
    # counts: float64 [4]; replicate the reference float32 arithmetic
    tp = counts.astype(np.float32)
    denom = (np.float32(2.0) * tp + np.float32(1e-6)).astype(np.float32)
    dice_per_class = (np.float32(2.0) * tp / denom).astype(np.float32)
    dice = np.float32(dice_per_class[1:].sum()) / np.float32(num_classes - 1)
    loss = np.float32(balance) * (np.float32(1.0) - dice ** np.float32(0.75))
    return np.float32(loss)


def kernel(**inputs):
    pred = np.ascontiguousarray(np.asarray(inputs["pred_labels"], dtype=np.int32))
    targ = np.ascontiguousarray(np.asarray(inputs["target_labels"], dtype=np.int32))
    balance = np.float32(np.asarray(inputs.get("balance", 1.0)))
    num_classes = int(np.asarray(inputs.get("num_classes", 4)))

    from concourse.bass_utils import run_bass_kernel_spmd

    nc = _get_nc()
    pred_sh = pred.reshape(NCORES, P, 1, TOT)
    targ_sh = targ.reshape(NCORES, P, 1, TOT)
    # interleave per partition row: [NCORES, P, 2, TOT]
    pt = np.concatenate([pred_sh, targ_sh], axis=2)
    in_maps = [{"pt": pt[i]} for i in range(NCORES)]
    trace = os.environ.get("DICE_TRACE", "") == "1"
    res = run_bass_kernel_spmd(
        nc, in_maps, core_ids=list(range(NCORES)), trace=trace
    )
    global last_results
    last_results = res

    counts = np.zeros(4, dtype=np.float64)
    for r in res.results:
        n1, n2, n3 = unpack_counts(r["out"])
        counts[1] += n1
        counts[2] += n2
        counts[3] += n3
    counts = np.rint(counts)
    return _dice_from_counts(counts, balance, num_classes)
